# revision 2
# baseline (speedup 1.0000x reference)
"""Trainium2 Bass kernel for the per-batch attention block.

Reference math (per batch b, with C=E=512, H=W=32 -> N=1024, heads=8, d=64):
    qkv = w_in @ x_flat                      # [3E, N]
    S_h = q_h^T k_h * heads**-0.5            # [N, N] per head
    P_h = softmax(S_h, axis=-1)
    o_h = v_h @ P_h^T                        # [d, N]
    out = w_out @ concat(o_h) + b_out + x_flat

Mapping: data-parallel over batch across 8 NeuronCores (B=8, one batch
element per core). Inside a core everything is computed in a transposed
"S^T" layout so the TensorEngine contracts along its partition axis with
no on-chip transposes:
  - q,k produced channel-major ([ch, pos]); v produced position-major
    (v^T = x^T @ w_v^T) with a ones-column per head interleaved.
  - S^T = k_h^T q_h (two heads packed in the PE array via 64-row tiling),
    softmax along partitions: exp on ScalarE with no max subtraction
    (logits are bounded ~|S|<30 -> safe in f32), denominators come for
    free as row 0 of the o-matmul via the ones-column.
  - o = (v^T)^T @ P^T accumulated over position chunks in PSUM.
  - normalize = reciprocal(DVE) + partition-broadcast + multiply (GpSimd),
    output projection with bias folded in as a K=1 matmul, residual add
    fused into the PSUM evacuation.
All matmul operands are float32r (TF32-like, full PE rate, ~1.5e-4 rel).
"""

import sys

if "/opt/trn_rl_repo" not in sys.path:
    sys.path.insert(0, "/opt/trn_rl_repo")

from contextlib import ExitStack, nullcontext

import numpy as np

import concourse.bass as bass
import concourse.tile as tile
from concourse import bacc, mybir
from concourse.bass_utils import run_bass_kernel_spmd

F32 = mybir.dt.float32
F32R = mybir.dt.float32r
EXP = mybir.ActivationFunctionType.Exp

C = 512
N = 1024
E = 512
HEADS = 8
D = 64
NH = D + 1  # ones column + 64 v-channels per head
SCALE = float(HEADS) ** -0.5
P = 128
N_CORES = 8


def _build(n_cores=N_CORES, use_row_tiling=False, reps=1):
    nc = bacc.Bacc(
        "TRN2", target_bir_lowering=False, debug=False, num_devices=n_cores
    )
    x_d = nc.dram_tensor("x", [C, N], F32R, kind="ExternalInput").ap()
    wqkT_d = nc.dram_tensor("wqkT", [C, 2 * E], F32R, kind="ExternalInput").ap()
    wvT_d = nc.dram_tensor("wvT", [C, E], F32R, kind="ExternalInput").ap()
    woutT_d = nc.dram_tensor("woutT", [E, C], F32R, kind="ExternalInput").ap()
    bias_d = nc.dram_tensor("bias", [1, C], F32R, kind="ExternalInput").ap()
    out_d = nc.dram_tensor("out", [C, N], F32, kind="ExternalOutput").ap()

    with tile.TileContext(nc) as tc, ExitStack() as ctx:
        consts = ctx.enter_context(tc.tile_pool(name="consts", bufs=1))
        qk_pool = ctx.enter_context(tc.tile_pool(name="qk", bufs=1))
        vt_pool = ctx.enter_context(tc.tile_pool(name="vt", bufs=1))
        osb_pool = ctx.enter_context(tc.tile_pool(name="osb", bufs=1))
        misc_pool = ctx.enter_context(tc.tile_pool(name="misc", bufs=2))

        # ---- load inputs (issue order = first-use order) ----------------
        xf = []
        wqkT = []
        wvT = []
        woutT = []
        for c in range(4):
            tx = consts.tile([P, N], F32R, tag=f"xf{c}", name=f"xf{c}")
            nc.sync.dma_start(tx[:], x_d[c * P : (c + 1) * P, :])
            xf.append(tx)
            tw = consts.tile([P, 2 * E], F32R, tag=f"wqkT{c}", name=f"wqkT{c}")
            nc.scalar.dma_start(tw[:], wqkT_d[c * P : (c + 1) * P, :])
            wqkT.append(tw)
        for c in range(4):
            t = consts.tile([P, E], F32R, tag=f"wvT{c}", name=f"wvT{c}")
            nc.sync.dma_start(t[:], wvT_d[c * P : (c + 1) * P, :])
            wvT.append(t)
        for e in range(4):
            t = consts.tile([P, C], F32R, tag=f"woutT{e}", name=f"woutT{e}")
            nc.scalar.dma_start(t[:], woutT_d[e * P : (e + 1) * P, :])
            woutT.append(t)
        bias_sb = consts.tile([1, C], F32R, tag="bias", name="bias_sb")
        nc.scalar.dma_start(bias_sb[:], bias_d[:])
        ones_f32 = consts.tile([1, 512], F32, tag="ones_f32", name="ones_f32")
        nc.vector.memset(ones_f32[:], 1.0)
        ones_row = consts.tile([1, 512], F32R, tag="ones", name="ones_row")
        nc.vector.tensor_copy(ones_row[:], ones_f32[:])
        ones_col_f32 = consts.tile([P, HEADS], F32, tag="ones_col", name="ones_col")
        nc.vector.memset(ones_col_f32[:], 1.0)

        # ---- phases B/C/D interleaved -----------------------------------
        # One shared 8-bank PSUM pool: tags s0/s1 (2 banks each) serve both
        # projection accumulators and S^T tiles; tags o0/o1 hold per-head
        # attention-output accumulators. Pair j's attention runs while the
        # next pair's q/k projections fill PE gaps (ACT is the steady-state
        # bottleneck).
        qk_sb = [None] * 8
        vt_sb = [None] * 8
        osb = []
        for j in range(4):
            t = osb_pool.tile([P, N], F32R, tag=f"osb{j}", name=f"osb{j}")
            osb.append(t)

        rep_ctx = (
            tc.For_i(0, reps, 1, hint_engines=(mybir.EngineType.PE,))
            if reps > 1
            else nullcontext()
        )
        with (
            tc.tile_pool(name="ps", bufs=1, space="PSUM") as ps,
            tc.tile_pool(name="pt", bufs=3) as pt_pool,
            tc.tile_pool(name="norm", bufs=1) as norm_pool,
            rep_ctx,
        ):

            def emit_B(m):
                psum = ps.tile([P, N], F32, tag=f"s{m % 2}", name=f"psB{m}")
                for c in range(4):
                    for ih in range(2):
                        nc.tensor.matmul(
                            psum[:, ih * 512 : (ih + 1) * 512],
                            wqkT[c][:, m * P : (m + 1) * P],
                            xf[c][:, ih * 512 : (ih + 1) * 512],
                            start=(c == 0),
                            stop=(c == 3),
                        )
                t = qk_pool.tile([P, N], F32R, tag=f"qk{m}", name=f"qk{m}")
                nc.vector.tensor_copy(t[:], psum[:])
                qk_sb[m] = t

            def emit_C(n):
                psum = ps.tile([P, E], F32, tag=f"s{n % 2}", name=f"psC{n}")
                for c in range(4):
                    nc.tensor.matmul(
                        psum[:],
                        xf[c][:, n * P : (n + 1) * P],
                        wvT[c][:],
                        start=(c == 0),
                        stop=(c == 3),
                    )
                # per-head layout: [ones, v0..v63] so the denominator row of
                # the o-matmul lands on PSUM partition 0
                t = vt_pool.tile([P, HEADS * NH], F32R, tag=f"vt{n}", name=f"vt{n}")
                t3 = t[:].rearrange("p (h d) -> p h d", h=HEADS)
                nc.vector.tensor_copy(
                    t3[:, :, 1:NH], psum[:].rearrange("p (h d) -> p h d", h=HEADS)
                )
                nc.vector.tensor_copy(
                    t3[:, :, 0:1],
                    ones_col_f32[:].rearrange("p (h o) -> p h o", o=1),
                )
                vt_sb[n] = t

            def emit_pair(j, mid_work=None):
                h0, h1 = 2 * j, 2 * j + 1
                qt = qk_sb[j]
                kt = qk_sb[4 + j]
                o_ps0 = ps.tile([NH, N], F32, tag="o0", name=f"o_ps0_{j}")
                o_ps1 = ps.tile([NH, N], F32, tag="o1", name=f"o_ps1_{j}")
                for a in range(8):
                    if a == 4 and mid_work is not None:
                        mid_work()
                    s_ps0 = ps.tile([P, N], F32, tag="s0", name=f"s_ps0_{j}_{a}")
                    s_ps1 = ps.tile([P, N], F32, tag="s1", name=f"s_ps1_{j}_{a}")
                    for ih in range(2):
                        sl = slice(ih * 512, (ih + 1) * 512)
                        nc.tensor.matmul(
                            s_ps0[:, sl],
                            kt[0:64, a * P : (a + 1) * P],
                            qt[0:64, sl],
                            start=True,
                            stop=True,
                            tile_position=(0, 0) if use_row_tiling else None,
                        )
                        nc.tensor.matmul(
                            s_ps1[:, sl],
                            kt[64:128, a * P : (a + 1) * P],
                            qt[64:128, sl],
                            start=True,
                            stop=True,
                            tile_position=(64, 0) if use_row_tiling else None,
                        )
                    pt0 = pt_pool.tile([P, N], F32R, tag="pt0", name=f"pt0_{j}_{a}")
                    pt1 = pt_pool.tile([P, N], F32R, tag="pt1", name=f"pt1_{j}_{a}")
                    nc.scalar.activation(pt0[:], s_ps0[:], EXP, scale=SCALE)
                    nc.scalar.activation(pt1[:], s_ps1[:], EXP, scale=SCALE)
                    for ih in range(2):
                        sl = slice(ih * 512, (ih + 1) * 512)
                        nc.tensor.matmul(
                            o_ps0[:, sl],
                            vt_sb[a][:, h0 * NH : (h0 + 1) * NH],
                            pt0[:, sl],
                            start=(a == 0),
                            stop=(a == 7),
                        )
                        nc.tensor.matmul(
                            o_ps1[:, sl],
                            vt_sb[a][:, h1 * NH : (h1 + 1) * NH],
                            pt1[:, sl],
                            start=(a == 0),
                            stop=(a == 7),
                        )

                # normalize: row 0 of o_ps holds the softmax denominator.
                # Evacuate both PSUM accumulators first (frees the o0/o1
                # slots for the next pair), then recip / broadcast /
                # multiply in SBUF (broadcast+mul on GpSimd).
                o_alls = []
                for slot, o_ps in ((0, o_ps0), (1, o_ps1)):
                    h = 2 * j + slot
                    o_all = norm_pool.tile(
                        [NH, N], F32, tag=f"oall{slot}", name=f"oall{h}"
                    )
                    nc.vector.tensor_copy(o_all[:], o_ps[:])
                    o_alls.append(o_all)
                for slot in (0, 1):
                    h = 2 * j + slot
                    o_all = o_alls[slot]
                    r_row = norm_pool.tile(
                        [1, N], F32, tag=f"rrow{slot}", name=f"rrow{h}"
                    )
                    nc.vector.reciprocal_approx_fast(r_row[:], o_all[0:1, :])
                    r_bc = norm_pool.tile(
                        [NH, N], F32, tag=f"rbc{slot}", name=f"rbc{h}"
                    )
                    nc.gpsimd.partition_broadcast(r_bc[:], r_row[:], channels=NH)
                    o_f32 = norm_pool.tile(
                        [NH, N], F32, tag=f"of32_{slot}", name=f"of32_{h}"
                    )
                    nc.gpsimd.tensor_mul(o_f32[:], o_all[:], r_bc[:])
                    o_r = norm_pool.tile(
                        [NH, N], F32R, tag=f"or{slot}", name=f"or{h}"
                    )
                    nc.vector.tensor_copy(o_r[:], o_f32[:])
                    base = (h % 2) * D
                    nc.sync.dma_start(osb[j][base : base + D, :], o_r[1:NH, :])

            emit_B(0)
            emit_B(4)
            for n in range(8):
                emit_C(n)
            for j in range(4):
                if j < 3:

                    def mid(jn=j + 1):
                        emit_B(jn)
                        emit_B(jn + 4)

                    emit_pair(j, mid_work=mid)
                else:
                    emit_pair(j)

            # ---- phase E: output projection + bias + residual -----------
            # e-outer accumulation into 4 live PSUM tiles (reusing the
            # attention psum tags) so early head pairs start the projection
            # while later pairs are still normalizing.
            psums = [
                ps.tile([P, N], F32, tag=t_, name=f"psE{m}")
                for m, t_ in enumerate(("s0", "s1", "o0", "o1"))
            ]
            for m in range(4):
                for ih in range(2):
                    sl = slice(ih * 512, (ih + 1) * 512)
                    nc.tensor.matmul(
                        psums[m][:, sl],
                        bias_sb[:, m * P : (m + 1) * P],
                        ones_row[:],
                        start=True,
                        stop=False,
                    )
            for e in range(4):
                for m in range(4):
                    for ih in range(2):
                        sl = slice(ih * 512, (ih + 1) * 512)
                        nc.tensor.matmul(
                            psums[m][:, sl],
                            woutT[e][:, m * P : (m + 1) * P],
                            osb[e][:, sl],
                            start=False,
                            stop=(e == 3),
                        )
            for m in range(4):
                out_sb = misc_pool.tile([P, N], F32, tag="outsb", name=f"out_sb{m}")
                for ih in range(2):
                    sl = slice(ih * 512, (ih + 1) * 512)
                    nc.vector.tensor_add(
                        out_sb[:, sl], psums[m][:, sl], xf[m][:, sl].bitcast(F32)
                    )
                    eng = (nc.sync, nc.scalar, nc.sync, nc.scalar)[m]
                    eng.dma_start(out_d[m * P : (m + 1) * P, sl], out_sb[:, sl])

    nc.compile()
    return nc


_CACHE = {}


def _get_nc(reps=1):
    key = reps
    if key not in _CACHE:
        _CACHE[key] = _build(reps=reps)
    return _CACHE[key]


def kernel(x, w_in, w_out, b_out, heads):
    x = np.asarray(x)
    w_in = np.asarray(w_in)
    w_out = np.asarray(w_out)
    b_out = np.asarray(b_out)
    B = x.shape[0]
    assert int(heads) == HEADS, f"kernel compiled for heads=8, got {heads}"
    assert x.shape == (B, C, 32, 32) and B == N_CORES

    xf = np.ascontiguousarray(x.reshape(B, C, N), dtype=np.float32)
    wqkT = np.ascontiguousarray(w_in[: 2 * E].T, dtype=np.float32)
    wvT = np.ascontiguousarray(w_in[2 * E :].T, dtype=np.float32)
    woutT = np.ascontiguousarray(w_out.T, dtype=np.float32)
    bias = np.ascontiguousarray(b_out.reshape(1, C), dtype=np.float32)
    in_maps = [
        {"x": xf[b], "wqkT": wqkT, "wvT": wvT, "woutT": woutT, "bias": bias}
        for b in range(B)
    ]

    nc = _get_nc()
    res = run_bass_kernel_spmd(nc, in_maps, core_ids=list(range(N_CORES)))
    out = np.stack([r["out"] for r in res.results])
    return out.reshape(B, C, 32, 32).astype(x.dtype, copy=False)
